# revision 10
# baseline (speedup 1.0000x reference)
"""Trainium2 Bass kernel for a 4-layer dense transformer encoder (nn_Encoder).

Model: B=4, S=1024, D=1024, H=16 heads (dh=64), L=4 layers, VOCAB=32000.
  h = emb[x]*sqrt(D) + pos
  per layer: q,k,v = h@W{q,k,v}+b; attn = softmax(q k^T/8 + mask*-1e9) v
             h = LN(h + attn@Wo+bo); h = LN(h + h@Wf+bf)

Sharding: 8 cores; core c handles batch c//2, sequence half c%2 (512 rows).
Activations live transposed on-chip: hT [D=1024 (8 tiles of 128 partitions),
S_local=512 free].  Per layer each core computes K^T and V for its own 512
rows, pair-AllGathers them with its batch partner (cores 2b, 2b+1), and runs
full attention for its 512 query rows over the full 1024-position K/V.
Matmuls run in float32r (TF32-like fp32 PE feed mode).

Self-contained: builds, compiles and runs via bass/PJRT; caches the compiled
executable at module level.
"""

import numpy as np

import concourse.bass as bass
import concourse.mybir as mybir
import concourse.tile as tile
from concourse import bacc
from concourse.masks import make_identity

F32 = mybir.dt.float32
F32R = mybir.dt.float32r
I32 = mybir.dt.int32
AF = mybir.ActivationFunctionType
ALU = mybir.AluOpType

N_CORES = 8
B, S, D, H, L, VOCAB = 4, 1024, 1024, 16, 4, 32000
DH = D // H  # 64
SL = S // 2  # 512 local rows per core
NT = D // 128  # 8 d-tiles
RT = SL // 128  # 4 local row tiles
EPS = 1e-6


def build_nc(num_layers=L, layer_reps=1, debug=False):
    nc = bacc.Bacc("TRN2", target_bir_lowering=False, debug=False, num_devices=N_CORES)

    xids = nc.dram_tensor("xids", [128, RT], I32, kind="ExternalInput")
    emb = nc.dram_tensor("emb", [VOCAB, D], F32, kind="ExternalInput")
    posT = nc.dram_tensor("posT", [D, SL], F32, kind="ExternalInput")
    maskc = nc.dram_tensor("maskc", [128, NT], F32, kind="ExternalInput")
    wq = nc.dram_tensor("wq", [L, D, D], F32, kind="ExternalInput")
    wk = nc.dram_tensor("wk", [L, D, D], F32, kind="ExternalInput")
    wv = nc.dram_tensor("wv", [L, D, D], F32, kind="ExternalInput")
    wo = nc.dram_tensor("wo", [L, D, D], F32, kind="ExternalInput")
    wf = nc.dram_tensor("wf", [L, D, D], F32, kind="ExternalInput")
    bqc = nc.dram_tensor("bqc", [L, 128, NT], F32, kind="ExternalInput")
    bkc = nc.dram_tensor("bkc", [L, 128, NT], F32, kind="ExternalInput")
    boc = nc.dram_tensor("boc", [L, 128, NT], F32, kind="ExternalInput")
    bfc = nc.dram_tensor("bfc", [L, 128, NT], F32, kind="ExternalInput")
    g1c = nc.dram_tensor("g1c", [L, 128, NT], F32, kind="ExternalInput")
    b1c = nc.dram_tensor("b1c", [L, 128, NT], F32, kind="ExternalInput")
    g2c = nc.dram_tensor("g2c", [L, 128, NT], F32, kind="ExternalInput")
    b2c = nc.dram_tensor("b2c", [L, 128, NT], F32, kind="ExternalInput")
    outT = nc.dram_tensor("outT", [D, SL], F32, kind="ExternalOutput")
    if debug:
        dbg_qT = nc.dram_tensor("dbg_qT", [D, SL], F32, kind="ExternalOutput")
        dbg_kT = nc.dram_tensor("dbg_kT", [D, S], F32, kind="ExternalOutput")
        dbg_vA = nc.dram_tensor("dbg_vA", [S, H * (DH + 1)], F32, kind="ExternalOutput")
        dbg_ctx = nc.dram_tensor("dbg_ctx", [D, SL], F32, kind="ExternalOutput")
        dbg_res = nc.dram_tensor("dbg_res", [D, SL], F32, kind="ExternalOutput")

    W = {"q": wq, "k": wk, "v": wv, "o": wo, "f": wf}
    groups = [[0, 1], [2, 3], [4, 5], [6, 7]]

    with tile.TileContext(nc) as tc:
        with (
            tc.tile_pool(name="persist", bufs=1) as pp,
            tc.tile_pool(name="h", bufs=1) as hp,
            tc.tile_pool(name="wbuf", bufs=1) as wp,
            tc.tile_pool(name="kv", bufs=1) as kvp,
            tc.tile_pool(name="work", bufs=1) as tp,
            tc.tile_pool(name="probs", bufs=3) as prp,
            tc.tile_pool(name="psum", bufs=4, space="PSUM") as psp,
            tc.tile_pool(name="psum_ctx", bufs=2, space="PSUM") as pcp,
            tc.tile_pool(name="dram", bufs=1, space="DRAM") as dp,
        ):
            # ---------- constants ----------
            ones128 = pp.tile([128, 128], F32R)
            nc.vector.memset(ones128[:].bitcast(F32), 1.0)
            ident = pp.tile([128, 128], F32)
            make_identity(nc, ident[:])
            eps_sb = pp.tile([128, 1], F32)
            nc.vector.memset(eps_sb[:], EPS)
            mask_sb = pp.tile([128, NT], F32)
            nc.sync.dma_start(mask_sb[:], maskc[:])
            nc.vector.tensor_scalar_mul(mask_sb[:], mask_sb[:], -1e9)
            xids_sb = pp.tile([128, RT], I32)
            nc.sync.dma_start(xids_sb[:], xids[:])

            # ---------- embedding: gather + transpose + scale + pos ----------
            hA = [
                hp.tile([128, SL], F32R, name=f"hA_{dt}", tag=f"hA{dt}", bufs=1)
                for dt in range(NT)
            ]
            for rt in range(RT):
                g_sb = tp.tile([128, D], F32, name="g", tag="g", bufs=1)
                nc.gpsimd.indirect_dma_start(
                    out=g_sb[:],
                    out_offset=None,
                    in_=emb[:],
                    in_offset=bass.IndirectOffsetOnAxis(
                        ap=xids_sb[:, rt : rt + 1], axis=0
                    ),
                )
                for dt in range(NT):
                    ps_t = psp.tile([128, 128], F32, name="ps_t", tag="mmS", bufs=4)
                    nc.tensor.transpose(
                        ps_t[:], g_sb[:, dt * 128 : (dt + 1) * 128], ident[:]
                    )
                    nc.scalar.activation(
                        hA[dt][:, rt * 128 : (rt + 1) * 128], ps_t[:], AF.Copy,
                        scale=32.0,
                    )
            for dt in range(NT):
                pos_sb = tp.tile([128, SL], F32, name="pos", tag="pos", bufs=1)
                nc.sync.dma_start(pos_sb[:], posT[dt * 128 : (dt + 1) * 128, :])
                nc.vector.tensor_add(hA[dt][:], hA[dt][:], pos_sb[:])

            # ---------- helpers ----------
            def load_w(name, l):
                tiles = []
                for kt in range(NT):
                    t = wp.tile([128, D], F32R, name=f"w_{name}_{kt}", tag=f"w{kt}", bufs=1)
                    nc.sync.dma_start(
                        t[:], W[name][l, kt * 128 : (kt + 1) * 128, :].bitcast(F32R)
                    )
                    tiles.append(t)
                return tiles

            def bias_col(t, dt):
                return t[:, dt : dt + 1]

            def load_bias(src, l):
                t = tp.tile([128, NT], F32, name="bias", tag="bias", bufs=4)
                nc.sync.dma_start(t[:], src[l])
                return t

            def layernorm(h_in, g_sbuf, b_sbuf, out_tag):
                ps_sum = psp.tile([128, SL], F32, name="ln_s", tag="mmS", bufs=4)
                ps_sq = psp.tile([128, SL], F32, name="ln_q", tag="mmS", bufs=4)
                for dt in range(NT):
                    nc.tensor.matmul(
                        ps_sum[:], ones128[:], h_in[dt][:],
                        start=(dt == 0), stop=(dt == NT - 1),
                    )
                for dt in range(NT):
                    sq = tp.tile([128, SL], F32R, name="sq", tag="sq", bufs=3)
                    nc.scalar.square(sq[:], h_in[dt][:])
                    nc.tensor.matmul(
                        ps_sq[:], ones128[:], sq[:],
                        start=(dt == 0), stop=(dt == NT - 1),
                    )
                m_b = tp.tile([128, SL], F32, name="m_b", tag="m_b", bufs=1)
                nc.scalar.activation(m_b[:], ps_sum[:], AF.Copy, scale=1.0 / D)
                ex2 = tp.tile([128, SL], F32, name="ex2", tag="ex2", bufs=1)
                nc.scalar.activation(ex2[:], ps_sq[:], AF.Copy, scale=1.0 / D)
                var = tp.tile([128, SL], F32, name="var", tag="var", bufs=1)
                nc.vector.tensor_mul(var[:], m_b[:], m_b[:])
                nc.vector.tensor_sub(var[:], ex2[:], var[:])
                std = tp.tile([128, SL], F32, name="std", tag="std", bufs=1)
                nc.scalar.activation(std[:], var[:], AF.Sqrt, bias=eps_sb[:, :1])
                rstd = tp.tile([128, SL], F32, name="rstd", tag="rstd", bufs=1)
                nc.vector.reciprocal(rstd[:], std[:])
                out = []
                for dt in range(NT):
                    o = hp.tile(
                        [128, SL], F32R, name=f"ln_o{dt}", tag=f"{out_tag}{dt}", bufs=1
                    )
                    nc.vector.tensor_sub(o[:], h_in[dt][:], m_b[:])
                    nc.vector.tensor_mul(o[:], o[:], rstd[:])
                    nc.vector.tensor_scalar(
                        o[:], o[:], bias_col(g_sbuf, dt), bias_col(b_sbuf, dt),
                        ALU.mult, ALU.add,
                    )
                    out.append(o)
                return out

            # ---------- layers ----------
            for rep in range(layer_reps):
                for l in range(num_layers):
                    ag_in = dp.tile([2 * SL * D], F32R, name=f"agin_{rep}_{l}")
                    ag_in_k = ag_in[0 : D * SL].rearrange("(p f) -> p f", p=D)
                    ag_in_v = ag_in[D * SL : 2 * D * SL].rearrange("(p f) -> p f", p=SL)

                    # --- K^T local ---
                    wk_t = load_w("k", l)
                    bk_sb = load_bias(bkc, l)
                    for mt in range(NT):
                        ps = psp.tile([128, SL], F32, name="ps_k", tag="mmS", bufs=4)
                        for kt in range(NT):
                            nc.tensor.matmul(
                                ps[:],
                                wk_t[kt][:, mt * 128 : (mt + 1) * 128],
                                hA[kt][:],
                                start=(kt == 0), stop=(kt == NT - 1),
                            )
                        kt_sb = tp.tile([128, SL], F32R, name="kt_sb", tag="aux", bufs=2, padded_shape=[128, D])
                        nc.scalar.activation(
                            kt_sb[:], ps[:], AF.Identity, bias=bias_col(bk_sb, mt)
                        )
                        nc.sync.dma_start(
                            ag_in_k[mt * 128 : (mt + 1) * 128, :], kt_sb[:]
                        )

                    # --- V local ---
                    wv_t = load_w("v", l)
                    for rt in range(RT):
                        v_sb = tp.tile([128, D], F32R, name="v_sb", tag="aux", bufs=2)
                        for nh in range(2):
                            ps = psp.tile([128, SL], F32, name="ps_v", tag="mmS", bufs=4)
                            for kt in range(NT):
                                nc.tensor.matmul(
                                    ps[:],
                                    hA[kt][:, rt * 128 : (rt + 1) * 128],
                                    wv_t[kt][:, nh * SL : (nh + 1) * SL],
                                    start=(kt == 0), stop=(kt == NT - 1),
                                )
                            nc.scalar.copy(v_sb[:, nh * SL : (nh + 1) * SL], ps[:])
                        nc.sync.dma_start(
                            ag_in_v[rt * 128 : (rt + 1) * 128, :], v_sb[:]
                        )

                    # --- pair AllGather (overlaps with q projection) ---
                    ag_out = dp.tile([2, 2 * SL * D], F32R, name=f"agout_{rep}_{l}")
                    nc.gpsimd.collective_compute(
                        "AllGather",
                        ALU.bypass,
                        replica_groups=groups,
                        ins=[ag_in[:].opt()],
                        outs=[ag_out[:].opt()],
                    )

                    # --- Q^T ---
                    wq_t = load_w("q", l)
                    bq_sb = load_bias(bqc, l)
                    qT = []
                    for mt in range(NT):
                        ps = psp.tile([128, SL], F32, name="ps_q", tag="mmS", bufs=4)
                        for kt in range(NT):
                            nc.tensor.matmul(
                                ps[:],
                                wq_t[kt][:, mt * 128 : (mt + 1) * 128],
                                hA[kt][:],
                                start=(kt == 0), stop=(kt == NT - 1),
                            )
                        q_sb = hp.tile([128, SL], F32R, name=f"qT{mt}", tag=f"qT{mt}", bufs=1)
                        nc.scalar.activation(
                            q_sb[:], ps[:], AF.Identity, bias=bias_col(bq_sb, mt)
                        )
                        qT.append(q_sb)

                    # --- read back full-sequence K^T and V(aug) ---
                    kT = []
                    for dt in range(NT):
                        t = kvp.tile([128, S], F32R, name=f"kT{dt}", tag=f"kT{dt}", bufs=1)
                        for r in range(2):
                            src = ag_out[r, 0 : D * SL].rearrange("(p f) -> p f", p=D)
                            nc.sync.dma_start(
                                t[:, r * SL : (r + 1) * SL],
                                src[dt * 128 : (dt + 1) * 128, :],
                            )
                        kT.append(t)
                    vA = []
                    for st in range(NT):  # S row tiles; rank r = st//RT
                        t = kvp.tile(
                            [128, H * (DH + 1)], F32R, name=f"vA{st}", tag=f"vA{st}", bufs=1
                        )
                        r, lrt = st // RT, st % RT
                        src = ag_out[r, D * SL : 2 * D * SL].rearrange(
                            "(p f) -> p f", p=SL
                        )[lrt * 128 : (lrt + 1) * 128, :].rearrange(
                            "p (h d) -> p h d", h=H
                        )
                        dst = t[:].rearrange("p (h e) -> p h e", h=H)
                        nc.sync.dma_start(dst[:, :, 0:DH], src)
                        nc.vector.memset(dst[:, :, DH : DH + 1].bitcast(F32), 1.0)
                        vA.append(t)

                    if debug and rep == 0 and l == 0:
                        for dt in range(NT):
                            nc.sync.dma_start(
                                dbg_qT[dt * 128 : (dt + 1) * 128, :], qT[dt][:].bitcast(F32)
                            )
                            nc.sync.dma_start(
                                dbg_kT[dt * 128 : (dt + 1) * 128, :], kT[dt][:].bitcast(F32)
                            )
                            nc.sync.dma_start(
                                dbg_vA[dt * 128 : (dt + 1) * 128, :], vA[dt][:].bitcast(F32)
                            )

                    # --- attention per head ---
                    ctxn = [
                        tp.tile([128, SL], F32R, name=f"ctxn{p}", tag=f"ctxn{p}", bufs=1)
                        for p in range(NT)
                    ]
                    for h in range(H):
                        qtile, qoff = h // 2, (h % 2) * DH
                        ps_c = pcp.tile([DH + 1, SL], F32, name="ps_c", tag="ctx", bufs=2)
                        for st in range(NT):
                            ps_s = psp.tile([128, SL], F32, name="ps_s", tag="mmS", bufs=4)
                            nc.tensor.matmul(
                                ps_s[:],
                                kT[qtile][qoff : qoff + DH, st * 128 : (st + 1) * 128],
                                qT[qtile][qoff : qoff + DH, :],
                                start=True, stop=True,
                            )
                            pr = prp.tile([128, SL], F32R, name="pr", tag="pr", bufs=3)
                            nc.scalar.activation(
                                pr[:], ps_s[:], AF.Exp,
                                bias=mask_sb[:, st : st + 1], scale=0.125,
                            )
                            nc.tensor.matmul(
                                ps_c[:],
                                vA[st][:, h * (DH + 1) : (h + 1) * (DH + 1)],
                                pr[:],
                                start=(st == 0), stop=(st == NT - 1),
                            )
                        den = tp.tile([1, SL], F32, name="den", tag="den", bufs=4)
                        nc.scalar.copy(den[:], ps_c[DH : DH + 1, :])
                        rec = tp.tile([1, SL], F32, name="rec", tag="rec", bufs=4)
                        nc.vector.reciprocal(rec[:], den[:])
                        recb = tp.tile([DH, SL], F32, name="recb", tag="recb", bufs=4)
                        nc.gpsimd.partition_broadcast(recb[:], rec[:])
                        nc.vector.tensor_mul(
                            ctxn[qtile][qoff : qoff + DH, :], ps_c[0:DH, :], recb[:]
                        )

                    # --- WO + residual into hA ---
                    wo_t = load_w("o", l)
                    bo_sb = load_bias(boc, l)
                    for mt in range(NT):
                        ps = psp.tile([128, SL], F32, name="ps_o", tag="mmS", bufs=4)
                        for kt in range(NT):
                            nc.tensor.matmul(
                                ps[:],
                                wo_t[kt][:, mt * 128 : (mt + 1) * 128],
                                ctxn[kt][:],
                                start=(kt == 0), stop=(kt == NT - 1),
                            )
                        attn_sb = tp.tile([128, SL], F32R, name="attn", tag="aux", bufs=2, padded_shape=[128, D])
                        nc.scalar.activation(
                            attn_sb[:], ps[:], AF.Identity, bias=bias_col(bo_sb, mt)
                        )
                        nc.vector.tensor_add(hA[mt][:], hA[mt][:], attn_sb[:])

                    if debug and rep == 0 and l == 0:
                        for dt in range(NT):
                            nc.sync.dma_start(
                                dbg_ctx[dt * 128 : (dt + 1) * 128, :], ctxn[dt][:].bitcast(F32)
                            )
                            nc.sync.dma_start(
                                dbg_res[dt * 128 : (dt + 1) * 128, :], hA[dt][:].bitcast(F32)
                            )

                    g1_sb = load_bias(g1c, l)
                    b1_sb = load_bias(b1c, l)
                    hB = layernorm(hA, g1_sb, b1_sb, "qT")

                    # --- FFN + residual into hB ---
                    wf_t = load_w("f", l)
                    bf_sb = load_bias(bfc, l)
                    h2 = []
                    for mt in range(NT):
                        ps = psp.tile([128, SL], F32, name="ps_f", tag="mmS", bufs=4)
                        for kt in range(NT):
                            nc.tensor.matmul(
                                ps[:],
                                wf_t[kt][:, mt * 128 : (mt + 1) * 128],
                                hB[kt][:],
                                start=(kt == 0), stop=(kt == NT - 1),
                            )
                        # ff + bf + residual -> fresh tiles (ctxn slots are dead now);
                        # hB must stay intact until all FFN matmuls have read it
                        t2 = tp.tile([128, SL], F32R, name=f"h2_{mt}", tag=f"ctxn{mt}", bufs=1)
                        nc.scalar.activation(
                            t2[:], ps[:], AF.Identity, bias=bias_col(bf_sb, mt)
                        )
                        nc.vector.tensor_add(t2[:], t2[:], hB[mt][:])
                        h2.append(t2)

                    g2_sb = load_bias(g2c, l)
                    b2_sb = load_bias(b2c, l)
                    hA = layernorm(h2, g2_sb, b2_sb, "hA")

            # ---------- output ----------
            for dt in range(NT):
                nc.sync.dma_start(
                    outT[dt * 128 : (dt + 1) * 128, :], hA[dt][:].bitcast(F32)
                )

    nc.compile()
    return nc


# ---------------------------------------------------------------------------
# PJRT SPMD runner (inlined so kernel.py is self-contained)
# ---------------------------------------------------------------------------
class SpmdRunner:
    def __init__(self, nc, n_cores):
        import jax
        from jax.sharding import Mesh, PartitionSpec
        from jax.experimental.shard_map import shard_map
        from concourse.bass2jax import (
            _bass_exec_p,
            install_neuronx_cc_hook,
            partition_id_tensor,
        )

        install_neuronx_cc_hook()
        self.jax = jax
        self.nc = nc
        self.n_cores = n_cores
        partition_name = (
            nc.partition_id_tensor.name if nc.partition_id_tensor else None
        )
        in_names, out_names, out_avals = [], [], []
        for alloc in nc.m.functions[0].allocations:
            if not isinstance(alloc, mybir.MemoryLocationSet):
                continue
            name = alloc.memorylocations[0].name
            if alloc.kind == "ExternalInput":
                if name != partition_name:
                    in_names.append(name)
            elif alloc.kind == "ExternalOutput":
                out_names.append(name)
                out_avals.append(
                    jax.core.ShapedArray(
                        tuple(alloc.tensor_shape), mybir.dt.np(alloc.dtype)
                    )
                )
        self.in_names, self.out_names, self.out_avals = in_names, out_names, out_avals
        n_params = len(in_names)
        all_in_names = list(in_names) + list(out_names)
        if partition_name is not None:
            all_in_names.append(partition_name)

        def _body(*args):
            operands = list(args)
            if partition_name is not None:
                operands.append(partition_id_tensor())
            outs = _bass_exec_p.bind(
                *operands,
                out_avals=tuple(out_avals),
                in_names=tuple(all_in_names),
                out_names=tuple(out_names),
                lowering_input_output_aliases=(),
                sim_require_finite=True,
                sim_require_nnan=True,
                nc=nc,
            )
            return tuple(outs)

        devices = jax.devices()[:n_cores]
        self.mesh = Mesh(np.asarray(devices), ("core",))
        n_outs = len(out_names)
        self.fn = jax.jit(
            shard_map(
                _body,
                mesh=self.mesh,
                in_specs=(PartitionSpec("core"),) * (n_params + n_outs),
                out_specs=(PartitionSpec("core"),) * n_outs,
                check_rep=False,
            ),
            keep_unused=True,
        )
        self.sharding = jax.sharding.NamedSharding(self.mesh, PartitionSpec("core"))

    def put_inputs(self, in_maps):
        n = self.n_cores
        concat = [
            np.concatenate([np.asarray(in_maps[c][name]) for c in range(n)], axis=0)
            for name in self.in_names
        ]
        zeros = [
            np.zeros((n * av.shape[0], *av.shape[1:]), av.dtype)
            for av in self.out_avals
        ]
        return [self.jax.device_put(x, self.sharding) for x in concat + zeros]

    def run(self, dev_args):
        outs = self.fn(*dev_args)
        self.jax.block_until_ready(outs)
        return outs

    def results(self, outs):
        return [
            {
                name: np.asarray(outs[i]).reshape(
                    self.n_cores, *self.out_avals[i].shape
                )[c]
                for i, name in enumerate(self.out_names)
            }
            for c in range(self.n_cores)
        ]


# ---------------------------------------------------------------------------
# host-side prep
# ---------------------------------------------------------------------------
def _position_embedding():
    # must match the reference bit-for-bit: same jax ops on the same backend
    import jax.numpy as jnp

    pos = jnp.arange(S, dtype=jnp.float32)[:, None]
    i = jnp.arange(D, dtype=jnp.float32)[None, :]
    angle_rates = jnp.power(10000.0, 2.0 * jnp.floor(i / 2.0) / float(D))
    ang = pos * angle_rates
    return np.asarray(
        jnp.concatenate([jnp.sin(ang[:, 0::2]), jnp.cos(ang[:, 1::2])], axis=-1),
        np.float32,
    )


def _bias_cols(b):
    # [L, D] -> [L, 128, NT] with [l, p, t] = b[l, t*128+p]
    return np.ascontiguousarray(np.asarray(b, np.float32).reshape(L, NT, 128).transpose(0, 2, 1))


def make_in_maps(x, mask, emb, wq, bq, wk, bk, wv, bv, wo, bo, wf, bf, g1, b1, g2, b2):
    x = np.asarray(x, np.int32)
    mask = np.asarray(mask, np.float32)
    emb = np.ascontiguousarray(np.asarray(emb, np.float32))
    pos_full = _position_embedding()
    wq, wk, wv, wo, wf = [
        np.ascontiguousarray(np.asarray(w, np.float32)) for w in (wq, wk, wv, wo, wf)
    ]
    # fold bv through wo:  (ctx + bv) @ wo + bo == ctx @ wo + (bv @ wo + bo)
    bo_eff = np.einsum("ld,lde->le", np.asarray(bv, np.float32), wo) + np.asarray(
        bo, np.float32
    )
    common = dict(
        emb=emb, wq=wq, wk=wk, wv=wv, wo=wo, wf=wf,
        bqc=_bias_cols(bq), bkc=_bias_cols(bk), boc=_bias_cols(bo_eff),
        bfc=_bias_cols(bf), g1c=_bias_cols(g1), b1c=_bias_cols(b1),
        g2c=_bias_cols(g2), b2c=_bias_cols(b2),
    )
    in_maps = []
    for c in range(N_CORES):
        b_i, half = c // 2, c % 2
        ids = x[b_i, half * SL : (half + 1) * SL]
        in_maps.append(
            dict(
                xids=np.ascontiguousarray(ids.reshape(RT, 128).T).astype(np.int32),
                posT=np.ascontiguousarray(pos_full[half * SL : (half + 1) * SL, :].T),
                maskc=np.ascontiguousarray(mask[b_i, 0, 0, :].reshape(NT, 128).T),
                **common,
            )
        )
    return in_maps


_CACHE = {}


def get_runner():
    if "runner" not in _CACHE:
        _CACHE["runner"] = SpmdRunner(build_nc(), N_CORES)
    return _CACHE["runner"]


def kernel(**inputs):
    runner = get_runner()
    in_maps = make_in_maps(**inputs)
    dev = runner.put_inputs(in_maps)
    outs = runner.run(dev)
    res = runner.results(outs)
    out = np.empty((B, S, D), np.float32)
    for c in range(N_CORES):
        b_i, half = c // 2, c % 2
        out[b_i, half * SL : (half + 1) * SL, :] = res[c]["outT"].T
    return out
